# revision 53
# baseline (speedup 1.0000x reference)
"""Trainium2 Bass kernel for nn_AttentiveModel (B=32,S=128,D=300,P=200,V=30000,C=3).

Data-parallel over batch across 8 NeuronCores (4 batch items per core, all
weights replicated). Activations kept transposed [features(part), rows(free)];
weights/emb are pre-rounded to bf16 on the host and DMA'd as bf16 (half the
bytes, no on-chip convert), so every matmul runs at the 1-col/cycle bf16 PE
rate; PSUM accumulation stays fp32.

All feature dims chunk uniformly by 100 (D=3x100, P=2x100, 4D=12x100,
4P=8x100), so each weight matrix loads with ONE packed DMA into a
[100, (K/100)*M] tile whose column slices are the k-chunk lhsT views; DMAs
spread across the sync/scalar HW-DGE rings (gather on gpsimd) so the load
phase doesn't serialize behind one queue.

dist-attention att2[b,i,j] = sum_p 1/(1+|q1[b,i,p]-q2[b,j,p]|) runs as ONE
custom DVE op per (b, j-block, p-chunk):
    w  = ABSOLUTE_DIFF(q1, q2) + 1        (broadcast 3D APs, j x i grid)
    r  = 1/w via BITWISE_NOT exponent-flip seed + 1 Newton step
(7 ALU stages, max rel err 1.7e-3 over w in [1,64]), output written bf16.
The p-partition sums are pair-packed: PSUM row jh carries j=2jh and j=2jh+1
side by side, so each sliding ones-column bf16 matmul covers two j's (N=256),
halving the p-sum matmul count; att1 lands on top via stride-2 lhsT views and
the softmaxes run per packed half. Each batch item's att1/softmax/compare-cat
chain is interleaved right after its p-sums so it hides under the next batch's
DVE stream.
"""

import sys
from contextlib import ExitStack

import numpy as np

for _p in ("/opt/trn_rl_repo",):
    if _p not in sys.path:
        sys.path.insert(0, _p)

import concourse.bass as bass
import concourse.tile as tile
from concourse.bacc import Bacc
from concourse import mybir
from concourse.bass_utils import run_bass_kernel_spmd
from concourse.masks import make_identity

# ---------------------------------------------------------------------------
# activation-table steering: keep Exp resolvable only from exp_and_others and
# Sigmoid only from sigmoid_and_others so the table-load pass settles on
# sigmoid (highway) -> exp (softmax/cmp) -> sigmoid (cmp-highway tail).
import concourse.hw_specs as _hw_specs

_orig_gat = _hw_specs.get_activation_tables
_GAT_CACHE = {}


def _steered_gat(module_arch):
    if module_arch not in _GAT_CACHE:
        tabs = _orig_gat(module_arch)
        A = mybir.ActivationFunctionType
        out = {}
        for name, funcs in tabs.items():
            if name != "natural_log_exp_and_others":
                funcs = funcs - {A.Exp, A.Ln}
            if name != "sigmoid_and_others":
                funcs = funcs - {A.Sigmoid}
            out[name] = funcs
        _GAT_CACHE[module_arch] = out
    return _GAT_CACHE[module_arch]


_hw_specs.get_activation_tables = _steered_gat
import concourse.bacc as _bacc_mod
if getattr(_bacc_mod, "get_activation_tables", None) is not None:
    _bacc_mod.get_activation_tables = _steered_gat

# ---------------------------------------------------------------------------
# custom DVE op: r = 1/(1 + |src0 - src1|), one instruction, 7 ALU stages.
import concourse.dve_ops as _dve_ops_mod
from concourse.dve_spec import Spec, Src0, Src1, C0, C1, Bin, AluOp as DveAluOp, One, lower as _dve_lower
from concourse.dve_spec import _has_src1 as _dve_has_src1
from concourse.dve_uop import DveOpSpec as _DveOpSpec

_ABSRECIP_NAME = "ABSDIFF_RECIP_ANT"
# Chebyshev pair tuned for the [-4.5,-4] interval of w*bitcast(~w); after one
# Newton step max rel err is 1.7e-3 for w in [1, 64].
_RECIP_C0 = -0.23549792
_RECIP_C1 = 2.0017324


def _absrecip_ref(in0, in1, c0, c1, c2):
    w = (np.abs(in0.astype(np.float32) - in1.astype(np.float32)) + np.float32(1.0)).astype(np.float32)
    nx = (~w.view(np.int32)).view(np.float32)
    y0 = (nx * np.float32(c0)).astype(np.float32)
    return (y0 * (np.float32(c1) - w * y0)).astype(np.float32)


def _register_absrecip():
    if _ABSRECIP_NAME in _dve_ops_mod._SUB_OPCODE_FOR_NAME:
        for op in _dve_ops_mod.OPS:
            if op.name == _ABSRECIP_NAME:
                return op
    row = _dve_ops_mod._CUSTOM_DVE_ROW_BASE + len(_dve_ops_mod.OPS)
    assert row < 0x20
    _dve_ops_mod._SUB_OPCODE_FOR_NAME[_ABSRECIP_NAME] = row
    d = Bin(DveAluOp.ABSOLUTE_DIFF, Src0, Src1)
    w = d + One
    nx = Bin(DveAluOp.BITWISE_NOT, w, w)
    y0 = nx * C0
    y1 = y0 * (C1 - w * y0)
    spec = Spec(body=y1, reference=_absrecip_ref)
    shas = {}
    for ver in ("v3", "v4"):
        tmp = _DveOpSpec(
            name=_ABSRECIP_NAME,
            opcode=row,
            uops=_dve_lower(spec, ver=ver),
            rd1_en=_dve_has_src1(spec),
        )
        shas[ver] = tmp.sha(ver)
    op = _dve_ops_mod.DveOp(_ABSRECIP_NAME, spec, subdim=False, uops_sha=shas)
    _dve_ops_mod.OPS.append(op)
    _dve_ops_mod.CUSTOM_DVE_SPECS[_ABSRECIP_NAME] = spec
    return op


ABSRECIP = _register_absrecip()

F32 = mybir.dt.float32
BF16 = mybir.dt.bfloat16
I32 = mybir.dt.int32
ALU = mybir.AluOpType
ACTF = mybir.ActivationFunctionType
AX = mybir.AxisListType

B, S, D, P, V, C = 32, 128, 300, 200, 30000, 3
NCORES = 8
BL = B // NCORES  # 4 batch items per core
ROWS = BL * S  # 512

KC = 100  # uniform feature chunk
CH_D = [(i * KC, KC) for i in range(3)]  # 300
CH_P = [(i * KC, KC) for i in range(2)]  # 200

JB = 32  # j-block size for att2 streaming buffers
NBLK = S // JB  # 8

WEIGHT_NAMES = [
    "hw1_Wh", "hw1_bh", "hw1_Wt", "hw1_bt",
    "hw2_Wh", "hw2_bh", "hw2_Wt", "hw2_bt",
    "mul_W1", "mul_b1", "mul_W2", "mul_b2",
    "dist_W1", "dist_b1", "dist_W2", "dist_b2",
    "cmp_W1", "cmp_b1", "cmp_W2", "cmp_b2",
    "chw1_Wh", "chw1_bh", "chw1_Wt", "chw1_bt",
    "chw2_Wh", "chw2_bh", "chw2_Wt", "chw2_bt",
    "agg_W1", "agg_b1", "agg_W2", "agg_b2",
    "out_W", "out_b",
]

W_SHAPES = {
    "hw1_Wh": [D, D], "hw1_bh": [D], "hw1_Wt": [D, D], "hw1_bt": [D],
    "hw2_Wh": [D, D], "hw2_bh": [D], "hw2_Wt": [D, D], "hw2_bt": [D],
    "mul_W1": [D, P], "mul_b1": [P], "mul_W2": [P, P], "mul_b2": [P],
    "dist_W1": [D, P], "dist_b1": [P], "dist_W2": [P, P], "dist_b2": [P],
    "cmp_W1": [4 * D, P], "cmp_b1": [P], "cmp_W2": [P, P], "cmp_b2": [P],
    "chw1_Wh": [P, P], "chw1_bh": [P], "chw1_Wt": [P, P], "chw1_bt": [P],
    "chw2_Wh": [P, P], "chw2_bh": [P], "chw2_Wt": [P, P], "chw2_bt": [P],
    "agg_W1": [4 * P, P], "agg_b1": [P], "agg_W2": [P, P], "agg_b2": [P],
    "out_W": [P, C], "out_b": [C],
}


def build_nc():
    nc = Bacc()

    io = {}
    io["x1"] = nc.declare_dram_parameter("x1", [BL, S], I32, isOutput=False)
    io["x2"] = nc.declare_dram_parameter("x2", [BL, S], I32, isOutput=False)
    io["emb"] = nc.declare_dram_parameter("emb", [V, D], BF16, isOutput=False)
    for n in WEIGHT_NAMES:
        dt = F32 if n.endswith(("bh", "bt", "b1", "b2", "_b")) else BF16
        io[n] = nc.declare_dram_parameter(n, W_SHAPES[n], dt, isOutput=False)
    io["yt"] = nc.declare_dram_parameter("yt", [C, BL], F32, isOutput=True)

    with ExitStack() as ctx:
        tc = ctx.enter_context(tile.TileContext(nc))
        _emit(ctx, nc, tc, io)
    nc.finalize()
    return nc


def _emit(ctx, nc, tc, io):
    wpool = ctx.enter_context(tc.tile_pool(name="wpool", bufs=1))
    const = ctx.enter_context(tc.tile_pool(name="const", bufs=1))
    persist = ctx.enter_context(tc.tile_pool(name="persist", bufs=1))
    work = ctx.enter_context(tc.tile_pool(name="work", bufs=1))
    upool = ctx.enter_context(tc.tile_pool(name="upool", bufs=4))
    small = ctx.enter_context(tc.tile_pool(name="small", bufs=5))

    pp_mm = ctx.enter_context(tc.tile_pool(name="pp_mm", bufs=2, space="PSUM"))
    pp_sim = ctx.enter_context(tc.tile_pool(name="pp_sim", bufs=1, space="PSUM"))
    pp_sim1 = ctx.enter_context(tc.tile_pool(name="pp_sim1", bufs=1, space="PSUM"))
    pp_tr = ctx.enter_context(tc.tile_pool(name="pp_tr", bufs=1, space="PSUM"))
    pp_sm = ctx.enter_context(tc.tile_pool(name="pp_sm", bufs=1, space="PSUM"))

    # DMA ring round-robin for bulk loads (only SP/Activation/GpSimd may issue)
    rings = [nc.sync, nc.scalar, nc.gpsimd]
    ring_i = [0]

    def ring():
        r = rings[ring_i[0] % len(rings)]
        ring_i[0] += 1
        return r

    # ---------------- constants first: make_identity runs on the gpsimd
    # queue, and the eT transposes need the identity as soon as the first
    # gather lands -- emitting it before the gathers keeps PE fed.
    ident = const.tile([128, 128], BF16, tag="ident", name="ident")
    make_identity(nc, ident[:, :])
    identf = const.tile([128, 128], F32, tag="identf", name="identf")
    make_identity(nc, identf[:, :])

    zbuf = const.tile([128, 64], BF16, tag="zbuf", name="zbuf")
    nc.vector.memset(zbuf[:, :], 0.0)
    nc.vector.memset(zbuf[:, 32:33], 1.0)

    # ---------------- embedding gather (idx DMAs lead) ----------------
    gpool_ctx = ExitStack()
    gpool = gpool_ctx.enter_context(tc.tile_pool(name="gpool", bufs=1))
    e_n = {}
    for side, xh in (("1", io["x1"]), ("2", io["x2"])):
        e_n[side] = []
        for b in range(BL):
            idx = gpool.tile([128, 1], I32, tag=f"idx{side}_{b}", name=f"idx{side}_{b}")
            nc.sync.dma_start(out=idx[:, :], in_=xh[b, :])
            e = gpool.tile([128, D], BF16, tag=f"e{side}_{b}", name=f"e{side}_{b}")
            nc.gpsimd.indirect_dma_start(
                out=e[:, :], out_offset=None, in_=io["emb"][:, :],
                in_offset=bass.IndirectOffsetOnAxis(ap=idx[:, :1], axis=0),
            )
            e_n[side].append(e)

    # ------- weights: bf16 in DRAM (host-rounded), one packed DMA each ------
    W = {}

    def load_w(name):
        h = io[name]
        K, M = h.shape
        nch = K // KC
        in_ap = bass.AP(tensor=h.tensor if hasattr(h, "tensor") else h[:, :].tensor,
                        offset=h[:, :].offset,
                        ap=[[M, KC], [KC * M, nch], [1, M]])
        t = wpool.tile([KC, nch * M], BF16, tag=f"w_{name}", name=f"w_{name}")
        nc.sync.dma_start(
            out=t[:, :].rearrange("p (c m) -> p c m", c=nch), in_=in_ap)
        return [t[:, i * M:(i + 1) * M] for i in range(nch)]

    def load_b(name):
        h = io[name]
        (M,) = h.shape
        tiles = []
        o = 0
        i = 0
        while o < M:
            c = min(KC, M - o)
            t = wpool.tile([c, 1], F32, tag=f"b_{name}_{i}", name=f"b_{name}_{i}")
            r = nc.sync if (o + ord(name[0])) % 2 else nc.gpsimd
            r.dma_start(out=t[:, :], in_=h[o:o + c])
            tiles.append(t)
            o += c
            i += 1
        return tiles

    for n in WEIGHT_NAMES:
        W[n] = load_b(n) if n.endswith(("bh", "bt", "b1", "b2", "_b")) else load_w(n)

    # ---------------- helpers ----------------
    def mm_apply(w_views, b_tiles, rhs_tiles, n_free, func, out_tiles, mch=None,
                 drain="scalar", ocs=None):
        """out = func(W.T @ rhs + b), transposed layout, bf16 in/out.

        w_views: k-chunk [KC, M] lhsT views; rhs_tiles: matching [KC, n_free]
        activation APs; out_tiles: m-chunked [mc, n_free]. drain="dve" moves a
        Relu drain onto the vector engine (relu(x+b) as one tensor_scalar) for
        phases where ScalarE is the busier engine."""
        M = w_views[0].shape[1]
        if mch is None:
            mch = [(i * KC, min(KC, M - i * KC)) for i in range((M + KC - 1) // KC)]
        for mi, (mo, mc) in enumerate(mch):
            ps = pp_mm.tile([128, n_free], F32, tag="mmout", name="mmout")
            for idx in range(len(w_views)):
                nc.tensor.matmul(
                    out=ps[:mc, :],
                    lhsT=w_views[idx][:, mo:mo + mc],
                    rhs=rhs_tiles[idx],
                    start=(idx == 0),
                    stop=(idx == len(w_views) - 1),
                )
            oap = (out_tiles[mi][:mc, ocs] if ocs is not None
                   else out_tiles[mi][:mc, :n_free])
            if drain == "dve" and func == ACTF.Relu:
                nc.vector.tensor_scalar(
                    out=oap, in0=ps[:mc, :],
                    scalar1=b_tiles[mi][:mc, :], scalar2=0.0,
                    op0=ALU.add, op1=ALU.max)
            else:
                nc.scalar.activation(
                    out=oap, in_=ps[:mc, :],
                    func=func, bias=b_tiles[mi][:mc, :], scale=1.0,
                )

    def highway(uniq, xt_tiles, wh, bh, wt, bt, feat, out_tiles, cs=None,
                hdrain="dve"):
        """out = x + t*(h-x), transposed layout, bf16, over columns cs."""
        nch = feat // KC
        if cs is None:
            cs = slice(0, ROWS)
        n = cs.stop - cs.start
        h_tiles = [work.tile([KC, ROWS], BF16, tag=f"hwh_{uniq}_{i}", name=f"hwh_{uniq}_{i}") for i in range(nch)]
        t_tiles = [work.tile([KC, ROWS], BF16, tag=f"hwt_{uniq}_{i}", name=f"hwt_{uniq}_{i}") for i in range(nch)]
        xs = [x[:KC, cs] if not isinstance(x, bass.AP) else x for x in xt_tiles]
        mm_apply(wh, bh, xs, n, ACTF.Relu, h_tiles, drain=hdrain, ocs=cs)
        mm_apply(wt, bt, xs, n, ACTF.Sigmoid, t_tiles, ocs=cs)
        for mi in range(nch):
            tmp = work.tile([KC, ROWS], BF16, tag=f"hwtmp_{uniq}_{mi}", name=f"hwtmp_{uniq}_{mi}")
            nc.vector.tensor_tensor(
                out=tmp[:, cs], in0=h_tiles[mi][:, cs], in1=xs[mi],
                op=ALU.subtract)
            nc.vector.tensor_tensor(
                out=tmp[:, cs], in0=tmp[:, cs], in1=t_tiles[mi][:, cs],
                op=ALU.mult)
            nc.vector.tensor_tensor(
                out=out_tiles[mi][:KC, cs], in0=tmp[:, cs], in1=xs[mi],
                op=ALU.add)

    # ---------------- e (bf16 straight from gather) -> transpose into eT ----
    eT = {}
    for side in ("1", "2"):
        eb = e_n[side]
        eT[side] = [persist.tile([KC, ROWS], BF16, tag=f"eT{side}_{i}", name=f"eT{side}_{i}")
                    for i in range(3)]
        for ki, (ko, kc) in enumerate(CH_D):
            for b in range(BL):
                ps = pp_tr.tile([128, 128], BF16, tag="tr", name="tr")
                nc.tensor.transpose(out=ps[:kc, :], in_=eb[b][:, ko:ko + kc],
                                    identity=ident[:, :])
                nc.scalar.activation(
                    out=eT[side][ki][:, b * S:(b + 1) * S], in_=ps[:kc, :],
                    func=ACTF.Copy)

    gpool_ctx.close()

    # ---------------- highway stack ----------------
    # eTh padded to 112 partitions (%16) so the normal-layout copies can run
    # as DMA transposes instead of PE transposes.
    ETH_P = 112
    eTh = {}
    for side in ("1", "2"):
        h1 = [work.tile([KC, ROWS], BF16, tag=f"hwy1_{i}", name=f"hwy1_{i}") for i in range(3)]
        highway("a", [t[:, :] for t in eT[side]],
                W["hw1_Wh"], W["hw1_bh"], W["hw1_Wt"], W["hw1_bt"], D, h1)
        eTh[side] = [persist.tile([ETH_P, ROWS], BF16, tag=f"eTh{side}_{i}", name=f"eTh{side}_{i}")
                     for i in range(3)]
        for t in eTh[side]:
            # partition offsets must be 32-aligned; rows 96:100 are rewritten
            # by the highway below, rows 100:112 stay zero (transpose padding)
            nc.vector.memset(t[96:ETH_P, :], 0.0)
        highway("b", [t[:, :] for t in h1],
                W["hw2_Wh"], W["hw2_bh"], W["hw2_Wt"], W["hw2_bt"], D, eTh[side])

    # ---------------- projections (dist first: att2 is the DVE long pole) ---
    def proj(prefix, side, dst_pool):
        dr = "dve" if prefix == "dist" else "scalar"
        z1 = [work.tile([KC, ROWS], BF16, tag=f"z1_{i}", name=f"z1_{i}")
              for i in range(2)]
        mm_apply(W[f"{prefix}_W1"], W[f"{prefix}_b1"], [t[:KC, :] for t in eTh[side]],
                 ROWS, ACTF.Relu, z1, drain=dr)
        out = [dst_pool.tile([KC, ROWS], BF16, tag=f"{prefix}T{side}_{i}", name=f"{prefix}T{side}_{i}")
               for i in range(2)]
        mm_apply(W[f"{prefix}_W2"], W[f"{prefix}_b2"], [t[:, :] for t in z1],
                 ROWS, ACTF.Relu, out, drain=dr)
        return out

    q1T = proj("dist", "1", persist)
    q2T = proj("dist", "2", persist)

    # normal-layout post-highway embeddings via DMA transpose (PE stays free);
    # chunk ki lives at column 112*ki, with cols [100:112) transpose padding.
    ehw_n = {}
    for side in ("1", "2"):
        ehw_n[side] = [persist.tile([128, 3 * ETH_P], BF16, tag=f"ehwn{side}_{b}", name=f"ehwn{side}_{b}")
                       for b in range(BL)]
        for ki in range(3):
            for b in range(BL):
                nc.sync.dma_start(
                    out=ehw_n[side][b][:, ETH_P * ki:ETH_P * (ki + 1)],
                    in_=eTh[side][ki][:ETH_P, b * S:(b + 1) * S],
                    transpose=True)

    # ---------------- att2 + att1 + softmax + compare-cat, per b -----------
    # mul projection first so att1/softmax for batch b can chase b's p-sums
    # while DVE streams the next batch's att2 blocks.
    p1T = proj("mul", "1", persist)
    p2T = proj("mul", "2", persist)

    catm = {s: {part: [persist.tile([KC, ROWS], BF16, tag=f"cat{s}{part}{ki}", name=f"cat{s}{part}{ki}")
                       for ki in range(3)]
                for part in ("b", "s", "m")} for s in ("1", "2")}

    def softmax_p(src_psum):
        """softmax along free dim of [128,128] psum; returns transposed
        probabilities [i-contraction, out-rows] in bf16."""
        mx = small.tile([128, 1], F32, tag="sm_mx", name="sm_mx")
        nc.vector.tensor_reduce(out=mx[:, :], in_=src_psum[:, :], axis=AX.X,
                                op=ALU.max, negate=True)
        esb = small.tile([128, S], BF16, tag="sm_e", name="sm_e")
        zs = small.tile([128, 1], F32, tag="sm_z", name="sm_z")
        nc.scalar.activation(out=esb[:, :], in_=src_psum[:, :], func=ACTF.Exp,
                             bias=mx[:, :], scale=1.0, accum_out=zs[:, :])
        rz = small.tile([128, 1], F32, tag="sm_rz", name="sm_rz")
        nc.vector.reciprocal(out=rz[:, :], in_=zs[:, :])
        pr = small.tile([128, S], BF16, tag="sm_p", name="sm_p")
        nc.vector.tensor_scalar(out=pr[:, :], in0=esb[:, :], scalar1=rz[:, :],
                                scalar2=None, op0=ALU.mult)
        pt_ps = pp_tr.tile([128, 128], BF16, tag="tr", name="tr")
        nc.tensor.transpose(out=pt_ps[:, :], in_=pr[:, :], identity=ident[:, :])
        pt = small.tile([128, S], BF16, tag="sm_pt", name="sm_pt")
        nc.scalar.activation(out=pt[:, :], in_=pt_ps[:, :], func=ACTF.Copy)
        return pt

    # last OFF_N j-blocks per b run on GpSimd(sub) + ScalarE(Abs/Ln/Exp);
    # their chains are emitted FIRST so the slow engines work ahead while DVE
    # streams the custom-op blocks, and their p-sums still land last in PE
    # order (no straggler stall).


    def q_bcast(ki, b, jb):
        q1b = q1T[ki][:KC, b * S:(b + 1) * S]
        in0 = bass.AP(tensor=q1b.tensor, offset=q1b.offset,
                      ap=[q1b.ap[0], [0, JB], q1b.ap[1]])
        q2b = q2T[ki][:KC, b * S + jb * JB:b * S + (jb + 1) * JB]
        in1 = bass.AP(tensor=q2b.tensor, offset=q2b.offset,
                      ap=[q2b.ap[0], q2b.ap[1], [0, S]])
        return in0, in1

    # p-sums pair-packed: PSUM row jh holds j=2*jh (cols 0:128) and j=2*jh+1
    # (cols 128:256), so each ones-column matmul covers TWO j's with N=256.
    # simT2all [64, 4*256]: batch b at cols [b*256, (b+1)*256); b0/b1 share
    # PSUM bank 1, b2/b3 bank 2 (start flags zero a whole bank row).
    def emit_psums(b, jb, r_hi, r_lo):
        for pi in range(JB // 2):
            jh = jb * (JB // 2) + pi
            js = slice(2 * pi * S, (2 * pi + 2) * S)
            g, rr = jh // 32, jh % 32
            nc.tensor.matmul(
                out=simT2all[32 * g:32 * g + 32, b * 256:(b + 1) * 256],
                lhsT=zbuf[:KC, 32 - rr:64 - rr], rhs=r_hi[:KC, js],
                start=(b in (0, 2) and rr == 0), stop=False,
                skip_group_check=True, tile_position=(0, 32 * g),
            )
            nc.tensor.matmul(
                out=simT2all[32 * g:32 * g + 32, b * 256:(b + 1) * 256],
                lhsT=zbuf[:KC, 32 - rr:64 - rr], rhs=r_lo[:KC, js],
                start=False, stop=False, skip_group_check=True,
                tile_position=(0, 32 * g),
            )

    def softmax_half(view64):
        """softmax along free dim of a [64,128] psum half; returns [128, 64]
        transposed probabilities (bf16) in a PSUM tile to drain strided."""
        mx = small.tile([64, 1], F32, tag="smh_mx", name="smh_mx")
        nc.vector.tensor_reduce(out=mx[:, :], in_=view64, axis=AX.X,
                                op=ALU.max, negate=True)
        esb = small.tile([64, S], BF16, tag="smh_e", name="smh_e")
        zs = small.tile([64, 1], F32, tag="smh_z", name="smh_z")
        nc.scalar.activation(out=esb[:, :], in_=view64, func=ACTF.Exp,
                             bias=mx[:, :], scale=1.0, accum_out=zs[:, :])
        rz = small.tile([64, 1], F32, tag="smh_rz", name="smh_rz")
        nc.vector.reciprocal(out=rz[:, :], in_=zs[:, :])
        pr = small.tile([64, S], BF16, tag="smh_p", name="smh_p")
        nc.vector.tensor_scalar(out=pr[:, :], in0=esb[:, :], scalar1=rz[:, :],
                                scalar2=None, op0=ALU.mult)
        pt_ps = pp_tr.tile([128, 64], BF16, tag="trh", name="trh")
        nc.tensor.transpose(out=pt_ps[:, :64], in_=pr[:, :],
                            identity=ident[:64, :64])
        return pt_ps

    # ------- tail (compare+chw+stats), split by half-batch columns ---------
    cmp1 = {s: [work.tile([KC, ROWS], BF16, tag=f"cmp1_{s}_{i}", name=f"cmp1_{s}_{i}")
                for i in range(2)] for s in ("1", "2")}
    v0t = {s: [work.tile([KC, ROWS], BF16, tag=f"v0_{s}_{i}", name=f"v0_{s}_{i}")
               for i in range(2)] for s in ("1", "2")}
    v1t = {s: [work.tile([KC, ROWS], BF16, tag=f"v1_{s}_{i}", name=f"v1_{s}_{i}")
               for i in range(2)] for s in ("1", "2")}
    vT = {s: [persist.tile([KC, ROWS], BF16, tag=f"vT{s}_{i}", name=f"vT{s}_{i}")
              for i in range(2)] for s in ("1", "2")}
    stats = [[persist.tile([KC, BL], BF16, tag=f"st{sect}_{i}", name=f"st{sect}_{i}")
              for i in range(2)] for sect in range(4)]
    STAT_OPS = (("1", ALU.max), ("2", ALU.max), ("1", ALU.add), ("2", ALU.add))

    def tail_half(hh, full=False):
        cs = slice(0, ROWS) if full else slice(hh * 2 * S, (hh + 1) * 2 * S)
        n = cs.stop - cs.start
        for side in ("1", "2"):
            for ki in range(3):
                e_sl = eTh[side][ki][:KC, cs]
                b_sl = catm[side]["b"][ki][:KC, cs]
                nc.vector.tensor_tensor(
                    out=catm[side]["s"][ki][:KC, cs], in0=e_sl, in1=b_sl,
                    op=ALU.subtract)
                nc.vector.tensor_tensor(
                    out=catm[side]["m"][ki][:KC, cs], in0=e_sl, in1=b_sl,
                    op=ALU.mult)
        # stage-major across the two independent sides so side 2's matmuls
        # fill side 1's drain/combine latency
        for mi, (mo, mc) in enumerate(CH_P):
            for side in ("1", "2"):
                rhs_list = [eTh[side][ki][:KC, cs] for ki in range(3)]
                for part in ("b", "s", "m"):
                    rhs_list += [catm[side][part][ki][:KC, cs] for ki in range(3)]
                ps = pp_mm.tile([128, n], F32, tag="mmout", name="mmout")
                for idx in range(12):
                    nc.tensor.matmul(
                        out=ps[:mc, :],
                        lhsT=W["cmp_W1"][idx][:, mo:mo + mc],
                        rhs=rhs_list[idx],
                        start=(idx == 0), stop=(idx == 11),
                    )
                nc.scalar.activation(
                    out=cmp1[side][mi][:mc, cs], in_=ps[:mc, :], func=ACTF.Relu,
                    bias=W["cmp_b1"][mi][:mc, :], scale=1.0,
                )
        for side in ("1", "2"):
            mm_apply(W["cmp_W2"], W["cmp_b2"], [t[:KC, cs] for t in cmp1[side]],
                     n, ACTF.Relu, v0t[side], drain="scalar", ocs=cs)
        for side in ("1", "2"):
            highway("c", [t[:KC, cs] for t in v0t[side]],
                    W["chw1_Wh"], W["chw1_bh"], W["chw1_Wt"],
                    W["chw1_bt"], P, v1t[side], cs=cs, hdrain="scalar")
            highway("d", [t[:KC, cs] for t in v1t[side]],
                    W["chw2_Wh"], W["chw2_bh"], W["chw2_Wt"],
                    W["chw2_bt"], P, vT[side], cs=cs, hdrain="scalar")
        for sect, (side, op) in enumerate(STAT_OPS):
            for ki in range(2):
                for b in (range(BL) if full else (2 * hh, 2 * hh + 1)):
                    with nc.allow_low_precision(reason="128-col pooling; DVE accumulates fp32 internally, only the final write is bf16"):
                        nc.vector.tensor_reduce(
                            out=stats[sect][ki][:KC, b:b + 1],
                            in_=vT[side][ki][:KC, b * S:(b + 1) * S],
                            axis=AX.X, op=op,
                        )

    simT2all = pp_sim.tile([64, BL * 256], F32, tag="simT2all", name="simT2all")
    for b in range(BL):
        for jb in range(NBLK):
            r_hi = upool.tile([128, JB * S], BF16, tag="r_hi", name="r_hi")
            r_lo = upool.tile([128, JB * S], BF16, tag="r_lo", name="r_lo")
            for ki, rt in ((0, r_hi), (1, r_lo)):
                in0, in1 = q_bcast(ki, b, jb)
                ro = rt[:KC, :].rearrange("p (j i) -> p j i", j=JB)
                nc.vector._custom_dve(
                    ABSRECIP, out=ro, in0=in0, in1=in1,
                    s0=_RECIP_C0, s1=_RECIP_C1,
                )
            emit_psums(b, jb, r_hi, r_lo)

        bs = slice(b * S, (b + 1) * S)
        # att1 on top: even/odd j columns of p2T as stride-2 lhsT views
        for ki in range(2):
            for h in range(2):
                p2b = p2T[ki][:KC, b * S:(b + 1) * S]
                cstep = p2b.ap[1][0]
                lhs = bass.AP(tensor=p2b.tensor, offset=p2b.offset + h * cstep,
                              ap=[p2b.ap[0], [2 * cstep, 64]])
                nc.tensor.matmul(
                    out=simT2all[:64, b * 256 + h * S:b * 256 + (h + 1) * S],
                    lhsT=lhs, rhs=p1T[ki][:KC, bs],
                    start=False, stop=(ki == 1 and h == 1),
                    skip_group_check=True,
                )

        # alpha path: per-half softmax over i, reassembled with stride-2 cols
        ptA = small.tile([128, S], BF16, tag="sm_ptA", name="sm_ptA")
        for h in range(2):
            pt_ps = softmax_half(simT2all[:64, b * 256 + h * S:b * 256 + (h + 1) * S])
            outA = ptA[:, :]
            dst = bass.AP(tensor=outA.tensor, offset=outA.offset + h * outA.ap[1][0],
                          ap=[outA.ap[0], [2 * outA.ap[1][0], 64]])
            nc.scalar.activation(out=dst, in_=pt_ps[:, :64], func=ACTF.Copy)

        # beta path: unpack sim[i, j] via two half transposes (strided drain)
        simn_sb = small.tile([128, S], F32, tag="simn_sb", name="simn_sb")
        for h in range(2):
            half_sb = small.tile([64, S], F32, tag="simh_sb", name="simh_sb")
            nc.scalar.activation(
                out=half_sb[:, :],
                in_=simT2all[:64, b * 256 + h * S:b * 256 + (h + 1) * S],
                func=ACTF.Copy)
            tr_ps = pp_sim1.tile([128, 64], F32, tag="sim", name="sim")
            nc.tensor.transpose(
                out=tr_ps[:, :64], in_=half_sb[:, :],
                identity=identf[:64, :64])
            outS = simn_sb[:, :]
            dst = bass.AP(tensor=outS.tensor, offset=outS.offset + h * outS.ap[1][0],
                          ap=[outS.ap[0], [2 * outS.ap[1][0], 64]])
            nc.scalar.activation(out=dst, in_=tr_ps[:, :64], func=ACTF.Copy)
        ptB = softmax_p(simn_sb[:, :])  # P^T [j, i] for beta

        for side, pt, eln in (("1", ptB, "2"), ("2", ptA[:, :], "1")):
            for ki, (ko, kc) in enumerate(CH_D):
                bt_ps = pp_sm.tile([128, S], F32, tag="psm", name="psm")
                nc.tensor.matmul(
                    out=bt_ps[:kc, :],
                    lhsT=ehw_n[eln][b][:, ETH_P * ki:ETH_P * ki + kc],
                    rhs=pt if isinstance(pt, bass.AP) else pt[:, :],
                    start=True, stop=True,
                )
                nc.scalar.activation(
                    out=catm[side]["b"][ki][:kc, bs], in_=bt_ps[:kc, :],
                    func=ACTF.Copy)

    tail_half(0, full=True)

    # ---------------- aggregate ----------------
    agg_rhs = [stats[s][ki][:KC, :] for s in range(4) for ki in range(2)]
    y1 = [persist.tile([KC, BL], BF16, tag=f"y1_{i}", name=f"y1_{i}") for i in range(2)]
    mm_apply(W["agg_W1"], W["agg_b1"], agg_rhs, BL, ACTF.Relu, y1, drain="dve")
    y2 = [persist.tile([KC, BL], BF16, tag=f"y2_{i}", name=f"y2_{i}") for i in range(2)]
    mm_apply(W["agg_W2"], W["agg_b2"], [t[:, :] for t in y1], BL, ACTF.Relu, y2, drain="dve")

    yt_ps = pp_sm.tile([128, BL], F32, tag="psm", name="psm")
    for ki in range(2):
        nc.tensor.matmul(
            out=yt_ps[:C, :], lhsT=W["out_W"][ki][:, :C],
            rhs=y2[ki][:KC, :], start=(ki == 0), stop=(ki == 1),
        )
    yt_sb = persist.tile([C, BL], F32, tag="yt_sb", name="yt_sb")
    nc.scalar.activation(out=yt_sb[:, :], in_=yt_ps[:C, :], func=ACTF.Identity,
                         bias=W["out_b"][0][:C, :], scale=1.0)
    nc.sync.dma_start(out=io["yt"][:, :], in_=yt_sb[:, :])


_NC_CACHE = {}


def _get_nc():
    if "nc" not in _NC_CACHE:
        _NC_CACHE["nc"] = build_nc()
    return _NC_CACHE["nc"]


def make_in_maps(inputs):
    """Shard full inputs into 8 per-core input maps (weights/emb as bf16)."""
    import ml_dtypes
    x1 = np.ascontiguousarray(np.asarray(inputs["x1"]).astype(np.int32))
    x2 = np.ascontiguousarray(np.asarray(inputs["x2"]).astype(np.int32))
    shared = {}
    for n in WEIGHT_NAMES + ["emb"]:
        a = np.ascontiguousarray(np.asarray(inputs[n]).astype(np.float32))
        if not n.endswith(("bh", "bt", "b1", "b2", "_b")):
            a = np.ascontiguousarray(a.astype(ml_dtypes.bfloat16))
        shared[n] = a
    in_maps = []
    for c in range(NCORES):
        m = dict(shared)
        m["x1"] = x1[c * BL:(c + 1) * BL]
        m["x2"] = x2[c * BL:(c + 1) * BL]
        in_maps.append(m)
    return in_maps


def kernel(**inputs):
    nc = _get_nc()
    in_maps = make_in_maps(inputs)
    res = run_bass_kernel_spmd(nc, in_maps, core_ids=list(range(NCORES)))
    return np.concatenate([np.asarray(r["yt"]).T for r in res.results], axis=0)


if __name__ == "__main__":
    nc = build_nc()
    print("built ok")


# revision 54
# speedup vs baseline: 1.0049x; 1.0049x over previous
"""Trainium2 Bass kernel for nn_AttentiveModel (B=32,S=128,D=300,P=200,V=30000,C=3).

Data-parallel over batch across 8 NeuronCores (4 batch items per core, all
weights replicated). Activations kept transposed [features(part), rows(free)];
weights/emb are pre-rounded to bf16 on the host and DMA'd as bf16 (half the
bytes, no on-chip convert), so every matmul runs at the 1-col/cycle bf16 PE
rate; PSUM accumulation stays fp32.

All feature dims chunk uniformly by 100 (D=3x100, P=2x100, 4D=12x100,
4P=8x100), so each weight matrix loads with ONE packed DMA into a
[100, (K/100)*M] tile whose column slices are the k-chunk lhsT views; DMAs
spread across the sync/scalar HW-DGE rings (gather on gpsimd) so the load
phase doesn't serialize behind one queue.

dist-attention att2[b,i,j] = sum_p 1/(1+|q1[b,i,p]-q2[b,j,p]|) runs as ONE
custom DVE op per (b, j-block, p-chunk):
    w  = ABSOLUTE_DIFF(q1, q2) + 1        (broadcast 3D APs, j x i grid)
    r  = 1/w via BITWISE_NOT exponent-flip seed + 1 Newton step
(7 ALU stages, max rel err 1.7e-3 over w in [1,64]), output written bf16.
The p-partition sums are pair-packed: PSUM row jh carries j=2jh and j=2jh+1
side by side, so each sliding ones-column bf16 matmul covers two j's (N=256),
halving the p-sum matmul count; att1 lands on top via stride-2 lhsT views and
the softmaxes run per packed half. Each batch item's att1/softmax/compare-cat
chain is interleaved right after its p-sums so it hides under the next batch's
DVE stream.
"""

import sys
from contextlib import ExitStack

import numpy as np

for _p in ("/opt/trn_rl_repo",):
    if _p not in sys.path:
        sys.path.insert(0, _p)

import concourse.bass as bass
import concourse.tile as tile
from concourse.bacc import Bacc
from concourse import mybir
from concourse.bass_utils import run_bass_kernel_spmd
from concourse.masks import make_identity

# ---------------------------------------------------------------------------
# activation-table steering: keep Exp resolvable only from exp_and_others and
# Sigmoid only from sigmoid_and_others so the table-load pass settles on
# sigmoid (highway) -> exp (softmax/cmp) -> sigmoid (cmp-highway tail).
import concourse.hw_specs as _hw_specs

_orig_gat = _hw_specs.get_activation_tables
_GAT_CACHE = {}


def _steered_gat(module_arch):
    if module_arch not in _GAT_CACHE:
        tabs = _orig_gat(module_arch)
        A = mybir.ActivationFunctionType
        out = {}
        for name, funcs in tabs.items():
            if name != "natural_log_exp_and_others":
                funcs = funcs - {A.Exp, A.Ln}
            if name != "sigmoid_and_others":
                funcs = funcs - {A.Sigmoid}
            out[name] = funcs
        _GAT_CACHE[module_arch] = out
    return _GAT_CACHE[module_arch]


_hw_specs.get_activation_tables = _steered_gat
import concourse.bacc as _bacc_mod
if getattr(_bacc_mod, "get_activation_tables", None) is not None:
    _bacc_mod.get_activation_tables = _steered_gat

# ---------------------------------------------------------------------------
# custom DVE op: r = 1/(1 + |src0 - src1|), one instruction, 7 ALU stages.
import concourse.dve_ops as _dve_ops_mod
from concourse.dve_spec import Spec, Src0, Src1, C0, C1, Bin, AluOp as DveAluOp, One, lower as _dve_lower
from concourse.dve_spec import _has_src1 as _dve_has_src1
from concourse.dve_uop import DveOpSpec as _DveOpSpec

_ABSRECIP_NAME = "ABSDIFF_RECIP_ANT"
# Chebyshev pair tuned for the [-4.5,-4] interval of w*bitcast(~w); after one
# Newton step max rel err is 1.7e-3 for w in [1, 64].
_RECIP_C0 = -0.23549792
_RECIP_C1 = 2.0017324


def _absrecip_ref(in0, in1, c0, c1, c2):
    w = (np.abs(in0.astype(np.float32) - in1.astype(np.float32)) + np.float32(1.0)).astype(np.float32)
    nx = (~w.view(np.int32)).view(np.float32)
    y0 = (nx * np.float32(c0)).astype(np.float32)
    return (y0 * (np.float32(c1) - w * y0)).astype(np.float32)


def _register_absrecip():
    if _ABSRECIP_NAME in _dve_ops_mod._SUB_OPCODE_FOR_NAME:
        for op in _dve_ops_mod.OPS:
            if op.name == _ABSRECIP_NAME:
                return op
    row = _dve_ops_mod._CUSTOM_DVE_ROW_BASE + len(_dve_ops_mod.OPS)
    assert row < 0x20
    _dve_ops_mod._SUB_OPCODE_FOR_NAME[_ABSRECIP_NAME] = row
    d = Bin(DveAluOp.ABSOLUTE_DIFF, Src0, Src1)
    w = d + One
    nx = Bin(DveAluOp.BITWISE_NOT, w, w)
    y0 = nx * C0
    y1 = y0 * (C1 - w * y0)
    spec = Spec(body=y1, reference=_absrecip_ref)
    shas = {}
    for ver in ("v3", "v4"):
        tmp = _DveOpSpec(
            name=_ABSRECIP_NAME,
            opcode=row,
            uops=_dve_lower(spec, ver=ver),
            rd1_en=_dve_has_src1(spec),
        )
        shas[ver] = tmp.sha(ver)
    op = _dve_ops_mod.DveOp(_ABSRECIP_NAME, spec, subdim=False, uops_sha=shas)
    _dve_ops_mod.OPS.append(op)
    _dve_ops_mod.CUSTOM_DVE_SPECS[_ABSRECIP_NAME] = spec
    return op


ABSRECIP = _register_absrecip()

F32 = mybir.dt.float32
BF16 = mybir.dt.bfloat16
I32 = mybir.dt.int32
ALU = mybir.AluOpType
ACTF = mybir.ActivationFunctionType
AX = mybir.AxisListType

B, S, D, P, V, C = 32, 128, 300, 200, 30000, 3
NCORES = 8
BL = B // NCORES  # 4 batch items per core
ROWS = BL * S  # 512

KC = 100  # uniform feature chunk
CH_D = [(i * KC, KC) for i in range(3)]  # 300
CH_P = [(i * KC, KC) for i in range(2)]  # 200

JB = 32  # j-block size for att2 streaming buffers
NBLK = S // JB  # 8

WEIGHT_NAMES = [
    "hw1_Wh", "hw1_bh", "hw1_Wt", "hw1_bt",
    "hw2_Wh", "hw2_bh", "hw2_Wt", "hw2_bt",
    "mul_W1", "mul_b1", "mul_W2", "mul_b2",
    "dist_W1", "dist_b1", "dist_W2", "dist_b2",
    "cmp_W1", "cmp_b1", "cmp_W2", "cmp_b2",
    "chw1_Wh", "chw1_bh", "chw1_Wt", "chw1_bt",
    "chw2_Wh", "chw2_bh", "chw2_Wt", "chw2_bt",
    "agg_W1", "agg_b1", "agg_W2", "agg_b2",
    "out_W", "out_b",
]

W_SHAPES = {
    "hw1_Wh": [D, D], "hw1_bh": [D], "hw1_Wt": [D, D], "hw1_bt": [D],
    "hw2_Wh": [D, D], "hw2_bh": [D], "hw2_Wt": [D, D], "hw2_bt": [D],
    "mul_W1": [D, P], "mul_b1": [P], "mul_W2": [P, P], "mul_b2": [P],
    "dist_W1": [D, P], "dist_b1": [P], "dist_W2": [P, P], "dist_b2": [P],
    "cmp_W1": [4 * D, P], "cmp_b1": [P], "cmp_W2": [P, P], "cmp_b2": [P],
    "chw1_Wh": [P, P], "chw1_bh": [P], "chw1_Wt": [P, P], "chw1_bt": [P],
    "chw2_Wh": [P, P], "chw2_bh": [P], "chw2_Wt": [P, P], "chw2_bt": [P],
    "agg_W1": [4 * P, P], "agg_b1": [P], "agg_W2": [P, P], "agg_b2": [P],
    "out_W": [P, C], "out_b": [C],
}


def build_nc():
    nc = Bacc()

    io = {}
    io["x1"] = nc.declare_dram_parameter("x1", [BL, S], I32, isOutput=False)
    io["x2"] = nc.declare_dram_parameter("x2", [BL, S], I32, isOutput=False)
    io["emb"] = nc.declare_dram_parameter("emb", [V, D], BF16, isOutput=False)
    for n in WEIGHT_NAMES:
        dt = F32 if n.endswith(("bh", "bt", "b1", "b2", "_b")) else BF16
        io[n] = nc.declare_dram_parameter(n, W_SHAPES[n], dt, isOutput=False)
    io["yt"] = nc.declare_dram_parameter("yt", [C, BL], F32, isOutput=True)

    with ExitStack() as ctx:
        tc = ctx.enter_context(tile.TileContext(nc))
        _emit(ctx, nc, tc, io)
    nc.finalize()
    return nc


def _emit(ctx, nc, tc, io):
    wpool = ctx.enter_context(tc.tile_pool(name="wpool", bufs=1))
    const = ctx.enter_context(tc.tile_pool(name="const", bufs=1))
    persist = ctx.enter_context(tc.tile_pool(name="persist", bufs=1))
    work = ctx.enter_context(tc.tile_pool(name="work", bufs=1))
    upool = ctx.enter_context(tc.tile_pool(name="upool", bufs=4))
    small = ctx.enter_context(tc.tile_pool(name="small", bufs=5))

    pp_mm = ctx.enter_context(tc.tile_pool(name="pp_mm", bufs=2, space="PSUM"))
    pp_sim = ctx.enter_context(tc.tile_pool(name="pp_sim", bufs=1, space="PSUM"))
    pp_sim1 = ctx.enter_context(tc.tile_pool(name="pp_sim1", bufs=1, space="PSUM"))
    pp_tr = ctx.enter_context(tc.tile_pool(name="pp_tr", bufs=1, space="PSUM"))
    pp_sm = ctx.enter_context(tc.tile_pool(name="pp_sm", bufs=1, space="PSUM"))

    # DMA ring round-robin for bulk loads (only SP/Activation/GpSimd may issue)
    rings = [nc.sync, nc.scalar, nc.gpsimd]
    ring_i = [0]

    def ring():
        r = rings[ring_i[0] % len(rings)]
        ring_i[0] += 1
        return r

    # ---------------- embedding gather first (idx DMAs lead) ----------------
    gpool_ctx = ExitStack()
    gpool = gpool_ctx.enter_context(tc.tile_pool(name="gpool", bufs=1))
    e_n = {}
    for side, xh in (("1", io["x1"]), ("2", io["x2"])):
        e_n[side] = []
        for b in range(BL):
            idx = gpool.tile([128, 1], I32, tag=f"idx{side}_{b}", name=f"idx{side}_{b}")
            nc.sync.dma_start(out=idx[:, :], in_=xh[b, :])
            e = gpool.tile([128, D], BF16, tag=f"e{side}_{b}", name=f"e{side}_{b}")
            nc.gpsimd.indirect_dma_start(
                out=e[:, :], out_offset=None, in_=io["emb"][:, :],
                in_offset=bass.IndirectOffsetOnAxis(ap=idx[:, :1], axis=0),
            )
            e_n[side].append(e)

    # ---------------- constants ----------------
    ident = const.tile([128, 128], BF16, tag="ident", name="ident")
    make_identity(nc, ident[:, :])
    identf = const.tile([128, 128], F32, tag="identf", name="identf")
    make_identity(nc, identf[:, :])

    zbuf = const.tile([128, 64], BF16, tag="zbuf", name="zbuf")
    nc.vector.memset(zbuf[:, :], 0.0)
    nc.vector.memset(zbuf[:, 32:33], 1.0)

    # ------- weights: bf16 in DRAM (host-rounded), one packed DMA each ------
    W = {}

    def load_w(name):
        h = io[name]
        K, M = h.shape
        nch = K // KC
        in_ap = bass.AP(tensor=h.tensor if hasattr(h, "tensor") else h[:, :].tensor,
                        offset=h[:, :].offset,
                        ap=[[M, KC], [KC * M, nch], [1, M]])
        t = wpool.tile([KC, nch * M], BF16, tag=f"w_{name}", name=f"w_{name}")
        nc.sync.dma_start(
            out=t[:, :].rearrange("p (c m) -> p c m", c=nch), in_=in_ap)
        return [t[:, i * M:(i + 1) * M] for i in range(nch)]

    def load_b(name):
        h = io[name]
        (M,) = h.shape
        tiles = []
        o = 0
        i = 0
        while o < M:
            c = min(KC, M - o)
            t = wpool.tile([c, 1], F32, tag=f"b_{name}_{i}", name=f"b_{name}_{i}")
            r = nc.sync if (o + ord(name[0])) % 2 else nc.gpsimd
            r.dma_start(out=t[:, :], in_=h[o:o + c])
            tiles.append(t)
            o += c
            i += 1
        return tiles

    for n in WEIGHT_NAMES:
        W[n] = load_b(n) if n.endswith(("bh", "bt", "b1", "b2", "_b")) else load_w(n)

    # ---------------- helpers ----------------
    def mm_apply(w_views, b_tiles, rhs_tiles, n_free, func, out_tiles, mch=None,
                 drain="scalar", ocs=None):
        """out = func(W.T @ rhs + b), transposed layout, bf16 in/out.

        w_views: k-chunk [KC, M] lhsT views; rhs_tiles: matching [KC, n_free]
        activation APs; out_tiles: m-chunked [mc, n_free]. drain="dve" moves a
        Relu drain onto the vector engine (relu(x+b) as one tensor_scalar) for
        phases where ScalarE is the busier engine."""
        M = w_views[0].shape[1]
        if mch is None:
            mch = [(i * KC, min(KC, M - i * KC)) for i in range((M + KC - 1) // KC)]
        for mi, (mo, mc) in enumerate(mch):
            ps = pp_mm.tile([128, n_free], F32, tag="mmout", name="mmout")
            for idx in range(len(w_views)):
                nc.tensor.matmul(
                    out=ps[:mc, :],
                    lhsT=w_views[idx][:, mo:mo + mc],
                    rhs=rhs_tiles[idx],
                    start=(idx == 0),
                    stop=(idx == len(w_views) - 1),
                )
            oap = (out_tiles[mi][:mc, ocs] if ocs is not None
                   else out_tiles[mi][:mc, :n_free])
            if drain == "dve" and func == ACTF.Relu:
                nc.vector.tensor_scalar(
                    out=oap, in0=ps[:mc, :],
                    scalar1=b_tiles[mi][:mc, :], scalar2=0.0,
                    op0=ALU.add, op1=ALU.max)
            else:
                nc.scalar.activation(
                    out=oap, in_=ps[:mc, :],
                    func=func, bias=b_tiles[mi][:mc, :], scale=1.0,
                )

    def highway(uniq, xt_tiles, wh, bh, wt, bt, feat, out_tiles, cs=None,
                hdrain="dve"):
        """out = x + t*(h-x), transposed layout, bf16, over columns cs."""
        nch = feat // KC
        if cs is None:
            cs = slice(0, ROWS)
        n = cs.stop - cs.start
        h_tiles = [work.tile([KC, ROWS], BF16, tag=f"hwh_{uniq}_{i}", name=f"hwh_{uniq}_{i}") for i in range(nch)]
        t_tiles = [work.tile([KC, ROWS], BF16, tag=f"hwt_{uniq}_{i}", name=f"hwt_{uniq}_{i}") for i in range(nch)]
        xs = [x[:KC, cs] if not isinstance(x, bass.AP) else x for x in xt_tiles]
        mm_apply(wh, bh, xs, n, ACTF.Relu, h_tiles, drain=hdrain, ocs=cs)
        mm_apply(wt, bt, xs, n, ACTF.Sigmoid, t_tiles, ocs=cs)
        for mi in range(nch):
            tmp = work.tile([KC, ROWS], BF16, tag=f"hwtmp_{uniq}_{mi}", name=f"hwtmp_{uniq}_{mi}")
            nc.vector.tensor_tensor(
                out=tmp[:, cs], in0=h_tiles[mi][:, cs], in1=xs[mi],
                op=ALU.subtract)
            nc.vector.tensor_tensor(
                out=tmp[:, cs], in0=tmp[:, cs], in1=t_tiles[mi][:, cs],
                op=ALU.mult)
            nc.vector.tensor_tensor(
                out=out_tiles[mi][:KC, cs], in0=tmp[:, cs], in1=xs[mi],
                op=ALU.add)

    # ---------------- e (bf16 straight from gather) -> transpose into eT ----
    eT = {}
    for side in ("1", "2"):
        eb = e_n[side]
        eT[side] = [persist.tile([KC, ROWS], BF16, tag=f"eT{side}_{i}", name=f"eT{side}_{i}")
                    for i in range(3)]
        for ki, (ko, kc) in enumerate(CH_D):
            for b in range(BL):
                ps = pp_tr.tile([128, 128], BF16, tag="tr", name="tr")
                nc.tensor.transpose(out=ps[:kc, :], in_=eb[b][:, ko:ko + kc],
                                    identity=ident[:, :])
                nc.scalar.activation(
                    out=eT[side][ki][:, b * S:(b + 1) * S], in_=ps[:kc, :],
                    func=ACTF.Copy)

    gpool_ctx.close()

    # ---------------- highway stack ----------------
    # eTh padded to 112 partitions (%16) so the normal-layout copies can run
    # as DMA transposes instead of PE transposes.
    ETH_P = 112
    eTh = {}
    for side in ("1", "2"):
        h1 = [work.tile([KC, ROWS], BF16, tag=f"hwy1_{i}", name=f"hwy1_{i}") for i in range(3)]
        highway("a", [t[:, :] for t in eT[side]],
                W["hw1_Wh"], W["hw1_bh"], W["hw1_Wt"], W["hw1_bt"], D, h1)
        eTh[side] = [persist.tile([ETH_P, ROWS], BF16, tag=f"eTh{side}_{i}", name=f"eTh{side}_{i}")
                     for i in range(3)]
        for t in eTh[side]:
            # partition offsets must be 32-aligned; rows 96:100 are rewritten
            # by the highway below, rows 100:112 stay zero (transpose padding)
            nc.vector.memset(t[96:ETH_P, :], 0.0)
        highway("b", [t[:, :] for t in h1],
                W["hw2_Wh"], W["hw2_bh"], W["hw2_Wt"], W["hw2_bt"], D, eTh[side])

    # ---------------- projections (dist first: att2 is the DVE long pole) ---
    def proj(prefix, side, dst_pool):
        dr = "dve" if prefix == "dist" else "scalar"
        z1 = [work.tile([KC, ROWS], BF16, tag=f"z1_{i}", name=f"z1_{i}")
              for i in range(2)]
        mm_apply(W[f"{prefix}_W1"], W[f"{prefix}_b1"], [t[:KC, :] for t in eTh[side]],
                 ROWS, ACTF.Relu, z1, drain=dr)
        out = [dst_pool.tile([KC, ROWS], BF16, tag=f"{prefix}T{side}_{i}", name=f"{prefix}T{side}_{i}")
               for i in range(2)]
        mm_apply(W[f"{prefix}_W2"], W[f"{prefix}_b2"], [t[:, :] for t in z1],
                 ROWS, ACTF.Relu, out, drain=dr)
        return out

    q1T = proj("dist", "1", persist)
    q2T = proj("dist", "2", persist)

    # normal-layout post-highway embeddings via DMA transpose (PE stays free);
    # chunk ki lives at column 112*ki, with cols [100:112) transpose padding.
    ehw_n = {}
    for side in ("1", "2"):
        ehw_n[side] = [persist.tile([128, 3 * ETH_P], BF16, tag=f"ehwn{side}_{b}", name=f"ehwn{side}_{b}")
                       for b in range(BL)]
        for ki in range(3):
            for b in range(BL):
                nc.sync.dma_start(
                    out=ehw_n[side][b][:, ETH_P * ki:ETH_P * (ki + 1)],
                    in_=eTh[side][ki][:ETH_P, b * S:(b + 1) * S],
                    transpose=True)

    # ---------------- att2 + att1 + softmax + compare-cat, per b -----------
    # mul projection first so att1/softmax for batch b can chase b's p-sums
    # while DVE streams the next batch's att2 blocks.
    p1T = proj("mul", "1", persist)
    p2T = proj("mul", "2", persist)

    catm = {s: {part: [persist.tile([KC, ROWS], BF16, tag=f"cat{s}{part}{ki}", name=f"cat{s}{part}{ki}")
                       for ki in range(3)]
                for part in ("b", "s", "m")} for s in ("1", "2")}

    def softmax_p(src_psum):
        """softmax along free dim of [128,128] psum; returns transposed
        probabilities [i-contraction, out-rows] in bf16."""
        mx = small.tile([128, 1], F32, tag="sm_mx", name="sm_mx")
        nc.vector.tensor_reduce(out=mx[:, :], in_=src_psum[:, :], axis=AX.X,
                                op=ALU.max, negate=True)
        esb = small.tile([128, S], BF16, tag="sm_e", name="sm_e")
        zs = small.tile([128, 1], F32, tag="sm_z", name="sm_z")
        nc.scalar.activation(out=esb[:, :], in_=src_psum[:, :], func=ACTF.Exp,
                             bias=mx[:, :], scale=1.0, accum_out=zs[:, :])
        rz = small.tile([128, 1], F32, tag="sm_rz", name="sm_rz")
        nc.vector.reciprocal(out=rz[:, :], in_=zs[:, :])
        pr = small.tile([128, S], BF16, tag="sm_p", name="sm_p")
        nc.vector.tensor_scalar(out=pr[:, :], in0=esb[:, :], scalar1=rz[:, :],
                                scalar2=None, op0=ALU.mult)
        pt_ps = pp_tr.tile([128, 128], BF16, tag="tr", name="tr")
        nc.tensor.transpose(out=pt_ps[:, :], in_=pr[:, :], identity=ident[:, :])
        pt = small.tile([128, S], BF16, tag="sm_pt", name="sm_pt")
        nc.scalar.activation(out=pt[:, :], in_=pt_ps[:, :], func=ACTF.Copy)
        return pt

    # last OFF_N j-blocks per b run on GpSimd(sub) + ScalarE(Abs/Ln/Exp);
    # their chains are emitted FIRST so the slow engines work ahead while DVE
    # streams the custom-op blocks, and their p-sums still land last in PE
    # order (no straggler stall).


    def q_bcast(ki, b, jb):
        q1b = q1T[ki][:KC, b * S:(b + 1) * S]
        in0 = bass.AP(tensor=q1b.tensor, offset=q1b.offset,
                      ap=[q1b.ap[0], [0, JB], q1b.ap[1]])
        q2b = q2T[ki][:KC, b * S + jb * JB:b * S + (jb + 1) * JB]
        in1 = bass.AP(tensor=q2b.tensor, offset=q2b.offset,
                      ap=[q2b.ap[0], q2b.ap[1], [0, S]])
        return in0, in1

    # p-sums pair-packed: PSUM row jh holds j=2*jh (cols 0:128) and j=2*jh+1
    # (cols 128:256), so each ones-column matmul covers TWO j's with N=256.
    # simT2all [64, 4*256]: batch b at cols [b*256, (b+1)*256); b0/b1 share
    # PSUM bank 1, b2/b3 bank 2 (start flags zero a whole bank row).
    def emit_psums(b, jb, r_hi, r_lo):
        for pi in range(JB // 2):
            jh = jb * (JB // 2) + pi
            js = slice(2 * pi * S, (2 * pi + 2) * S)
            g, rr = jh // 32, jh % 32
            nc.tensor.matmul(
                out=simT2all[32 * g:32 * g + 32, b * 256:(b + 1) * 256],
                lhsT=zbuf[:KC, 32 - rr:64 - rr], rhs=r_hi[:KC, js],
                start=(b in (0, 2) and rr == 0), stop=False,
                skip_group_check=True, tile_position=(0, 32 * g),
            )
            nc.tensor.matmul(
                out=simT2all[32 * g:32 * g + 32, b * 256:(b + 1) * 256],
                lhsT=zbuf[:KC, 32 - rr:64 - rr], rhs=r_lo[:KC, js],
                start=False, stop=False, skip_group_check=True,
                tile_position=(0, 32 * g),
            )

    def softmax_half(view64):
        """softmax along free dim of a [64,128] psum half; returns [128, 64]
        transposed probabilities (bf16) in a PSUM tile to drain strided."""
        mx = small.tile([64, 1], F32, tag="smh_mx", name="smh_mx")
        nc.vector.tensor_reduce(out=mx[:, :], in_=view64, axis=AX.X,
                                op=ALU.max, negate=True)
        esb = small.tile([64, S], BF16, tag="smh_e", name="smh_e")
        zs = small.tile([64, 1], F32, tag="smh_z", name="smh_z")
        nc.scalar.activation(out=esb[:, :], in_=view64, func=ACTF.Exp,
                             bias=mx[:, :], scale=1.0, accum_out=zs[:, :])
        rz = small.tile([64, 1], F32, tag="smh_rz", name="smh_rz")
        nc.vector.reciprocal(out=rz[:, :], in_=zs[:, :])
        pr = small.tile([64, S], BF16, tag="smh_p", name="smh_p")
        nc.vector.tensor_scalar(out=pr[:, :], in0=esb[:, :], scalar1=rz[:, :],
                                scalar2=None, op0=ALU.mult)
        pt_ps = pp_tr.tile([128, 64], BF16, tag="trh", name="trh")
        nc.tensor.transpose(out=pt_ps[:, :64], in_=pr[:, :],
                            identity=ident[:64, :64])
        return pt_ps

    # ------- tail (compare+chw+stats), split by half-batch columns ---------
    cmp1 = {s: [work.tile([KC, ROWS], BF16, tag=f"cmp1_{s}_{i}", name=f"cmp1_{s}_{i}")
                for i in range(2)] for s in ("1", "2")}
    v0t = {s: [work.tile([KC, ROWS], BF16, tag=f"v0_{s}_{i}", name=f"v0_{s}_{i}")
               for i in range(2)] for s in ("1", "2")}
    v1t = {s: [work.tile([KC, ROWS], BF16, tag=f"v1_{s}_{i}", name=f"v1_{s}_{i}")
               for i in range(2)] for s in ("1", "2")}
    vT = {s: [persist.tile([KC, ROWS], BF16, tag=f"vT{s}_{i}", name=f"vT{s}_{i}")
              for i in range(2)] for s in ("1", "2")}
    stats = [[persist.tile([KC, BL], BF16, tag=f"st{sect}_{i}", name=f"st{sect}_{i}")
              for i in range(2)] for sect in range(4)]
    STAT_OPS = (("1", ALU.max), ("2", ALU.max), ("1", ALU.add), ("2", ALU.add))

    def tail_half(hh, full=False):
        cs = slice(0, ROWS) if full else slice(hh * 2 * S, (hh + 1) * 2 * S)
        n = cs.stop - cs.start
        for side in ("1", "2"):
            for ki in range(3):
                e_sl = eTh[side][ki][:KC, cs]
                b_sl = catm[side]["b"][ki][:KC, cs]
                nc.vector.tensor_tensor(
                    out=catm[side]["s"][ki][:KC, cs], in0=e_sl, in1=b_sl,
                    op=ALU.subtract)
                nc.vector.tensor_tensor(
                    out=catm[side]["m"][ki][:KC, cs], in0=e_sl, in1=b_sl,
                    op=ALU.mult)
        # stage-major across the two independent sides so side 2's matmuls
        # fill side 1's drain/combine latency
        for mi, (mo, mc) in enumerate(CH_P):
            for side in ("1", "2"):
                rhs_list = [eTh[side][ki][:KC, cs] for ki in range(3)]
                for part in ("b", "s", "m"):
                    rhs_list += [catm[side][part][ki][:KC, cs] for ki in range(3)]
                ps = pp_mm.tile([128, n], F32, tag="mmout", name="mmout")
                for idx in range(12):
                    nc.tensor.matmul(
                        out=ps[:mc, :],
                        lhsT=W["cmp_W1"][idx][:, mo:mo + mc],
                        rhs=rhs_list[idx],
                        start=(idx == 0), stop=(idx == 11),
                    )
                nc.scalar.activation(
                    out=cmp1[side][mi][:mc, cs], in_=ps[:mc, :], func=ACTF.Relu,
                    bias=W["cmp_b1"][mi][:mc, :], scale=1.0,
                )
        for side in ("1", "2"):
            mm_apply(W["cmp_W2"], W["cmp_b2"], [t[:KC, cs] for t in cmp1[side]],
                     n, ACTF.Relu, v0t[side], drain="scalar", ocs=cs)
        for side in ("1", "2"):
            highway("c", [t[:KC, cs] for t in v0t[side]],
                    W["chw1_Wh"], W["chw1_bh"], W["chw1_Wt"],
                    W["chw1_bt"], P, v1t[side], cs=cs, hdrain="scalar")
            highway("d", [t[:KC, cs] for t in v1t[side]],
                    W["chw2_Wh"], W["chw2_bh"], W["chw2_Wt"],
                    W["chw2_bt"], P, vT[side], cs=cs, hdrain="scalar")
        for sect, (side, op) in enumerate(STAT_OPS):
            for ki in range(2):
                for b in (range(BL) if full else (2 * hh, 2 * hh + 1)):
                    with nc.allow_low_precision(reason="128-col pooling; DVE accumulates fp32 internally, only the final write is bf16"):
                        nc.vector.tensor_reduce(
                            out=stats[sect][ki][:KC, b:b + 1],
                            in_=vT[side][ki][:KC, b * S:(b + 1) * S],
                            axis=AX.X, op=op,
                        )

    simT2all = pp_sim.tile([64, BL * 256], F32, tag="simT2all", name="simT2all")
    for b in range(BL):
        for jb in range(NBLK):
            r_hi = upool.tile([128, JB * S], BF16, tag="r_hi", name="r_hi")
            r_lo = upool.tile([128, JB * S], BF16, tag="r_lo", name="r_lo")
            for ki, rt in ((0, r_hi), (1, r_lo)):
                in0, in1 = q_bcast(ki, b, jb)
                ro = rt[:KC, :].rearrange("p (j i) -> p j i", j=JB)
                nc.vector._custom_dve(
                    ABSRECIP, out=ro, in0=in0, in1=in1,
                    s0=_RECIP_C0, s1=_RECIP_C1,
                )
            emit_psums(b, jb, r_hi, r_lo)

        bs = slice(b * S, (b + 1) * S)
        # att1 on top: even/odd j columns of p2T as stride-2 lhsT views
        for ki in range(2):
            for h in range(2):
                p2b = p2T[ki][:KC, b * S:(b + 1) * S]
                cstep = p2b.ap[1][0]
                lhs = bass.AP(tensor=p2b.tensor, offset=p2b.offset + h * cstep,
                              ap=[p2b.ap[0], [2 * cstep, 64]])
                nc.tensor.matmul(
                    out=simT2all[:64, b * 256 + h * S:b * 256 + (h + 1) * S],
                    lhsT=lhs, rhs=p1T[ki][:KC, bs],
                    start=False, stop=(ki == 1 and h == 1),
                    skip_group_check=True,
                )

        # alpha path: per-half softmax over i, reassembled with stride-2 cols
        ptA = small.tile([128, S], BF16, tag="sm_ptA", name="sm_ptA")
        for h in range(2):
            pt_ps = softmax_half(simT2all[:64, b * 256 + h * S:b * 256 + (h + 1) * S])
            outA = ptA[:, :]
            dst = bass.AP(tensor=outA.tensor, offset=outA.offset + h * outA.ap[1][0],
                          ap=[outA.ap[0], [2 * outA.ap[1][0], 64]])
            nc.scalar.activation(out=dst, in_=pt_ps[:, :64], func=ACTF.Copy)

        # beta path: unpack sim[i, j] via two half transposes (strided drain)
        simn_sb = small.tile([128, S], F32, tag="simn_sb", name="simn_sb")
        for h in range(2):
            half_sb = small.tile([64, S], F32, tag="simh_sb", name="simh_sb")
            nc.scalar.activation(
                out=half_sb[:, :],
                in_=simT2all[:64, b * 256 + h * S:b * 256 + (h + 1) * S],
                func=ACTF.Copy)
            tr_ps = pp_sim1.tile([128, 64], F32, tag="sim", name="sim")
            nc.tensor.transpose(
                out=tr_ps[:, :64], in_=half_sb[:, :],
                identity=identf[:64, :64])
            outS = simn_sb[:, :]
            dst = bass.AP(tensor=outS.tensor, offset=outS.offset + h * outS.ap[1][0],
                          ap=[outS.ap[0], [2 * outS.ap[1][0], 64]])
            nc.scalar.activation(out=dst, in_=tr_ps[:, :64], func=ACTF.Copy)
        ptB = softmax_p(simn_sb[:, :])  # P^T [j, i] for beta

        for side, pt, eln in (("1", ptB, "2"), ("2", ptA[:, :], "1")):
            for ki, (ko, kc) in enumerate(CH_D):
                bt_ps = pp_sm.tile([128, S], F32, tag="psm", name="psm")
                nc.tensor.matmul(
                    out=bt_ps[:kc, :],
                    lhsT=ehw_n[eln][b][:, ETH_P * ki:ETH_P * ki + kc],
                    rhs=pt if isinstance(pt, bass.AP) else pt[:, :],
                    start=True, stop=True,
                )
                nc.scalar.activation(
                    out=catm[side]["b"][ki][:kc, bs], in_=bt_ps[:kc, :],
                    func=ACTF.Copy)

    tail_half(0, full=True)

    # ---------------- aggregate ----------------
    agg_rhs = [stats[s][ki][:KC, :] for s in range(4) for ki in range(2)]
    y1 = [persist.tile([KC, BL], BF16, tag=f"y1_{i}", name=f"y1_{i}") for i in range(2)]
    mm_apply(W["agg_W1"], W["agg_b1"], agg_rhs, BL, ACTF.Relu, y1, drain="dve")
    y2 = [persist.tile([KC, BL], BF16, tag=f"y2_{i}", name=f"y2_{i}") for i in range(2)]
    mm_apply(W["agg_W2"], W["agg_b2"], [t[:, :] for t in y1], BL, ACTF.Relu, y2, drain="dve")

    yt_ps = pp_sm.tile([128, BL], F32, tag="psm", name="psm")
    for ki in range(2):
        nc.tensor.matmul(
            out=yt_ps[:C, :], lhsT=W["out_W"][ki][:, :C],
            rhs=y2[ki][:KC, :], start=(ki == 0), stop=(ki == 1),
        )
    yt_sb = persist.tile([C, BL], F32, tag="yt_sb", name="yt_sb")
    nc.scalar.activation(out=yt_sb[:, :], in_=yt_ps[:C, :], func=ACTF.Identity,
                         bias=W["out_b"][0][:C, :], scale=1.0)
    nc.sync.dma_start(out=io["yt"][:, :], in_=yt_sb[:, :])


_NC_CACHE = {}


def _get_nc():
    if "nc" not in _NC_CACHE:
        _NC_CACHE["nc"] = build_nc()
    return _NC_CACHE["nc"]


def make_in_maps(inputs):
    """Shard full inputs into 8 per-core input maps (weights/emb as bf16)."""
    import ml_dtypes
    x1 = np.ascontiguousarray(np.asarray(inputs["x1"]).astype(np.int32))
    x2 = np.ascontiguousarray(np.asarray(inputs["x2"]).astype(np.int32))
    shared = {}
    for n in WEIGHT_NAMES + ["emb"]:
        a = np.ascontiguousarray(np.asarray(inputs[n]).astype(np.float32))
        if not n.endswith(("bh", "bt", "b1", "b2", "_b")):
            a = np.ascontiguousarray(a.astype(ml_dtypes.bfloat16))
        shared[n] = a
    in_maps = []
    for c in range(NCORES):
        m = dict(shared)
        m["x1"] = x1[c * BL:(c + 1) * BL]
        m["x2"] = x2[c * BL:(c + 1) * BL]
        in_maps.append(m)
    return in_maps


def kernel(**inputs):
    nc = _get_nc()
    in_maps = make_in_maps(inputs)
    res = run_bass_kernel_spmd(nc, in_maps, core_ids=list(range(NCORES)))
    return np.concatenate([np.asarray(r["yt"]).T for r in res.results], axis=0)


if __name__ == "__main__":
    nc = build_nc()
    print("built ok")
